# revision 44
# baseline (speedup 1.0000x reference)
"""BertScore model kernel for Trainium2 (8 NeuronCores, SPMD, length-specialized).

Reference: cosine-normalized per-layer token reps, per-(layer,batch)
similarity matrix dots = h1 @ h2^T (L1 x L2, contraction D=1024), ragged
max over valid rows/cols + means -> s1,s2, F1 harmonic mean -> (B,NL)
features, BatchNorm over batch, linear head -> (B,).

Strategy: the 256 (layer,batch) units are independent until the host-side
BatchNorm. Units are grouped into 32 SPMD "slots" of 8 (one per core) with
similar (len1,len2); the device program is compiled for the actual lengths
(slot shape = max lens over its 8 units, rounded to 16 for the dual-fp8
LDWEIGHTS stride-alignment ISA rule), so only the valid ragged region is
transferred and computed. Host-side replicate-padding (row/col len-1 copied
into the padded tail) keeps every max exact with no masking. Inputs are fp8
e4m3 (end-to-end rel err ~8e-3 vs the 2e-2 gate; DMA is the roofline and
fp8 halves it); matmuls run in DoubleRow perf mode (two 128-deep K-tiles
per instruction, 0.5 cyc/row).

Per slot (stationary = the shorter of h1/h2, halving LDWEIGHTS cost):
4*n_st DoubleRow matmuls -> ACT copies PSUM to SBUF as f16 (both stationary
tiles into one [128,2,L] tile) -> ONE merged DVE free-axis max over the
moving index (both tiles in a single instruction; DVE reduce fixed cost
~165ns dominates the short data phase, so halving the instruction count
matters) -> PE transposes (f16, software-pipelined one slot behind the
matmuls so the in-order PE queue never stalls on the ACT copy) -> one
merged DVE max over the stationary index. Garbage rows in merged-reduce
output columns land only in positions the host epilogue discards. Row/col
max vectors accumulate in SBUF f16 columns; two small DMAs out at the end.
Slots execute largest-first (minimal compute tail) and are DMA'd in
multi-slot chunks (first chunks small for fast pipeline fill) laid out so
each partition reads one contiguous run per chunk (~330 GB/s measured).
Host epilogue: means over valid prefixes, F1, BatchNorm, linear head.

Measured on trn2 (8 cores, NTFF profile of the single NEFF execution):
~44.7-45.2 us (median 45.0) in the final configuration, vs 295 us for the
staged baseline and 131 us for its single-shot profile. Chunk size trades
HWDGE descriptor-generation exposure (~2.6 us fixed per chunk DMA, 128
descriptors) against pipeline granularity; 12KB/partition hides generation
under the ~5 us transfer while keeping the fill acceptable.
"""
import os
import numpy as np

NL, B, L1, L2, D = 4, 64, 256, 256, 1024
NCORES = 8
NUNITS = NL * B           # 256 independent (layer, batch) units
NSLOTS = NUNITS // NCORES  # 32 slots, one unit per core each
KT = D // 128             # contraction tiles
BN_EPS = 1e-8
LOGIT_SCALE = 1.0

CHUNK_BYTES = int(os.environ.get("BSM_CHUNK", str(12 * 1024)))  # per partition

_CACHE = {}


def _plan(len1, len2):
    """Group units into slots; chunk slots for DMA; exec order big-first."""
    l1u = np.repeat(len1[None, :], NL, 0).ravel()   # unit u = l*B + b
    l2u = np.repeat(len2[None, :], NL, 0).ravel()
    nit = -(-l1u // 128)
    key = nit * 10**9 + l2u * 10**3 + l1u
    order_units = np.argsort(-key)                  # big first
    groups = order_units.reshape(NSLOTS, NCORES)
    # dual-fp8 LDWEIGHTS requires 16B-aligned k-tile strides
    shapes = np.stack([
        np.array([-16 * (-l1u[g].max() // 16) for g in groups]),
        np.array([-16 * (-l2u[g].max() // 16) for g in groups])], axis=1)
    order = np.argsort(-(shapes[:, 0] + shapes[:, 1]), kind="stable")
    # chunk consecutive exec-order slots: per-partition run per slot is
    # 8*(L1s+L2s) bytes. First chunks are small so compute starts early
    # (pipeline fill), later ones large (fewer DMA issues).
    ramp = [4 * 1024, 6 * 1024, 8 * 1024]
    chunks = []
    cur, cur_bytes = [], 0
    for s in order:
        cap = ramp[len(chunks)] if len(chunks) < len(ramp) else CHUNK_BYTES
        run = 8 * int(shapes[s][0] + shapes[s][1])
        if cur and cur_bytes + run > cap:
            chunks.append(cur)
            cur, cur_bytes = [], 0
        cur.append(int(s))
        cur_bytes += run
    if cur:
        chunks.append(cur)
    return {"groups": groups, "shapes": shapes, "order": order,
            "chunks": chunks}


def _build(shapes, chunks, plan):
    import concourse.bacc as bacc
    import concourse.bass as bass
    import concourse.mybir as mybir
    import concourse.tile as tile

    f32 = mybir.dt.float32
    f16 = mybir.dt.float16
    fp8 = mybir.dt.float8e4
    SWI = mybir.MatmulPerfMode.DoubleRowSwInterleave
    DR = mybir.MatmulPerfMode.DoubleRow

    # DRAM layout: chunk-major; within a chunk partition p holds one
    # contiguous run = concat over the chunk's slots of (8 d-rows of h1,
    # 8 d-rows of h2), each slot contributing 8*(L1s+L2s) bytes.
    chunk_off, chunk_run = [], []
    slot_in_chunk = {}
    off = 0
    for ci, ch in enumerate(chunks):
        run = 0
        for s in ch:
            slot_in_chunk[s] = (ci, run)
            run += 8 * int(shapes[s][0] + shapes[s][1])
        chunk_off.append(off)
        chunk_run.append(run)
        off += 128 * run
    TOT = off

    # RM: one f16 column per (slot, it); CM: one per (slot, jt).
    rm_col, cm_col = {}, {}
    nc1 = nc2 = 0
    for ch in chunks:
        for s in ch:
            L1s, L2s = int(shapes[s][0]), int(shapes[s][1])
            for it in range(-(-L1s // 128)):
                rm_col[(s, it)] = nc1
                nc1 += 1
            for jt in range(-(-L2s // 128)):
                cm_col[(s, jt)] = nc2
                nc2 += 1

    nc = bacc.Bacc("TRN2", target_bir_lowering=False, debug=False,
                   num_devices=NCORES)
    xin = nc.dram_tensor("xin", [TOT], fp8, kind="ExternalInput")
    rmd = nc.dram_tensor("rm", [128, nc1], f16, kind="ExternalOutput")
    cmd = nc.dram_tensor("cm", [128, nc2], f16, kind="ExternalOutput")
    xap = xin.ap()

    # quarters: contiguous chunk groups; each gets its own accumulator
    # tiles, DMA'd out as soon as its reduces are done (overlapped drain)
    NQ = 4
    nch = len(chunks)
    chunk_q = [min(NQ - 1, ci * NQ // nch) for ci in range(nch)]
    slot_q = {}
    for ci, ch in enumerate(chunks):
        for s in ch:
            slot_q[s] = chunk_q[ci]
    q_rm, q_cm = {}, {}   # quarter -> (col_lo, col_hi)
    for ci, ch in enumerate(chunks):
        q = chunk_q[ci]
        for s in ch:
            for t in range(-(-int(shapes[s][0]) // 128)):
                c = rm_col[(s, t)]
                lo, hi = q_rm.get(q, (c, c + 1))
                q_rm[q] = (min(lo, c), max(hi, c + 1))
            for t in range(-(-int(shapes[s][1]) // 128)):
                c = cm_col[(s, t)]
                lo, hi = q_cm.get(q, (c, c + 1))
                q_cm[q] = (min(lo, c), max(hi, c + 1))
    last_slot_of_q = {}
    for ci, ch in enumerate(chunks):
        last_slot_of_q[chunk_q[ci]] = ch[-1]
    last_slot_q = {s: q for q, s in last_slot_of_q.items()}

    with tile.TileContext(nc) as tc:
        from concourse.masks import make_identity
        with tc.tile_pool(name="consts", bufs=1) as consts, \
             tc.tile_pool(name="io", bufs=4) as io, \
             tc.tile_pool(name="dsbp", bufs=6) as dsbp, \
             tc.tile_pool(name="accp", bufs=1) as accp, \
             tc.tile_pool(name="ps", bufs=4, space="PSUM") as ps, \
             tc.tile_pool(name="psT", bufs=4, space="PSUM") as psT:

            ident = consts.tile([128, 128], f16)
            make_identity(nc, ident)
            RM = accp.tile([128, nc1], f16)
            CM = accp.tile([128, nc2], f16)

            vmax = mybir.AluOpType.max
            X = mybir.AxisListType.X
            IDENT = mybir.ActivationFunctionType.Identity

            MAXRUN = max(chunk_run)

            def emit_transposes(st):
                """Transpose phase of a slot: dT blocks + moving-side max."""
                s, Lst, Lmv, dsb, ilens, tr_acc, lc = st
                n_mv = -(-Lmv // 128)
                dT = psT.tile([128, 2, L1], f16, tag="dT")
                for tt in range(n_mv):
                    j0 = tt * 128
                    jlen = min(128, Lmv - j0)
                    for t, ilen in enumerate(ilens):
                        nc.tensor.transpose(
                            out=dT[:jlen, tt, t * 128:t * 128 + ilen],
                            in_=dsb[:ilen, t, j0:j0 + jlen],
                            identity=ident[:ilen, :ilen])
                if n_mv == 2:
                    # one merged reduce; tt=1 rows past jlen are garbage and
                    # discarded host-side (reduction is per-partition)
                    nc.vector.tensor_reduce(
                        out=tr_acc[:, lc:lc + 2], in_=dT[:, :, :Lst],
                        axis=X, op=vmax)
                else:
                    jlen = min(128, Lmv)
                    nc.vector.tensor_reduce(
                        out=tr_acc[:jlen, lc:lc + 1], in_=dT[:jlen, 0, :Lst],
                        axis=X, op=vmax)

            pending = None   # software pipeline: transposes lag one slot
            for ci, ch in enumerate(chunks):
                blk = io.tile([128, MAXRUN], fp8, tag="io")
                run = chunk_run[ci]
                # overlap the first two descriptor generations: ACT is idle
                # during pipeline fill and has its own HWDGE queue
                deng = nc.scalar if ci == 1 else nc.sync
                deng.dma_start(
                    out=blk[:, :run],
                    in_=bass.AP(tensor=xap.tensor, offset=chunk_off[ci],
                                ap=[[run, 128], [1, run]]))
                for s in ch:
                    L1s, L2s = int(shapes[s][0]), int(shapes[s][1])
                    soff = slot_in_chunk[s][1]
                    # stationary = shorter side (LDWEIGHTS cost ~ 8*Lst);
                    # free-axis reduce covers the stationary index,
                    # transposes cover the moving index.
                    if L1s <= L2s:
                        Lst, Lmv = L1s, L2s
                        fr_acc, frc = RM, rm_col[(s, 0)]
                        tr_acc, trc = CM, cm_col[(s, 0)]
                    else:
                        Lst, Lmv = L2s, L1s
                        fr_acc, frc = CM, cm_col[(s, 0)]
                        tr_acc, trc = RM, rm_col[(s, 0)]
                    stv = blk[:, soff:soff + 8 * Lst].rearrange(
                        "p (q i) -> p q i", q=8)
                    mvv = blk[:, soff + 8 * Lst:soff + 8 * (Lst + Lmv)
                              ].rearrange("p (q j) -> p q j", q=8)
                    n_st = -(-Lst // 128)

                    dsb = dsbp.tile([128, 2, L2], f16, tag="dsb")
                    ilens = []
                    for t in range(n_st):
                        i0 = t * 128
                        ilen = min(128, Lst - i0)
                        dps = ps.tile([128, L2], f32, tag="dots")
                        for k in range(0, KT, 2):
                            nc.tensor.matmul(
                                out=dps[:ilen, :Lmv],
                                lhsT=stv[:, k:k + 2, i0:i0 + ilen],
                                rhs=mvv[:, k:k + 2, :],
                                start=(k == 0), stop=(k == KT - 2),
                                perf_mode=DR)
                        nc.scalar.activation(
                            out=dsb[:ilen, t, :Lmv], in_=dps[:ilen, :Lmv],
                            func=IDENT)
                        ilens.append(ilen)
                    if n_st == 2:
                        # merged reduce; tile-1 rows past its ilen are
                        # garbage and discarded host-side
                        nc.vector.tensor_reduce(
                            out=fr_acc[:, frc:frc + 2],
                            in_=dsb[:, :, :Lmv], axis=X, op=vmax)
                    else:
                        nc.vector.tensor_reduce(
                            out=fr_acc[:ilens[0], frc:frc + 1],
                            in_=dsb[:ilens[0], 0, :Lmv], axis=X, op=vmax)

                    if pending is not None:
                        emit_transposes(pending)
                    pending = (s, Lst, Lmv, dsb, ilens, tr_acc, trc)
            if pending is not None:
                emit_transposes(pending)
            nc.sync.dma_start(out=rmd.ap(), in_=RM)
            nc.scalar.dma_start(out=cmd.ap(), in_=CM)

    nc.finalize()
    return nc, rm_col, cm_col


def _get_plan_nc(len1, len2):
    key = (tuple(len1.tolist()), tuple(len2.tolist()), CHUNK_BYTES)
    if key not in _CACHE:
        plan = _plan(len1, len2)
        nc, rm_col, cm_col = _build(plan["shapes"], plan["chunks"], plan)
        _CACHE[key] = (plan, nc, rm_col, cm_col)
    return _CACHE[key]


def _host_prep(reps1, reps2, len1, len2, plan):
    """Normalize, fp8-cast, replicate-pad, pack per-core chunk buffers."""
    import ml_dtypes
    np_in = ml_dtypes.float8_e4m3

    def prep(r, lens, L):
        r = np.asarray(r, dtype=np.float32)
        n = np.sqrt(np.einsum('lbid,lbid->lbi', r, r))
        h = r / n[..., None]
        idx = np.minimum(np.arange(L)[None, :], (lens - 1)[:, None])  # (B, L)
        h = np.take_along_axis(h, idx[None, :, :, None], axis=2)
        return np.ascontiguousarray(h.transpose(0, 1, 3, 2)).astype(np_in)

    h1t = prep(reps1, len1, L1)   # (NL, B, D, L)
    h2t = prep(reps2, len2, L2)

    def swi_pack(h, Lst):
        """[1024, Lst] d-major -> [128, 8*Lst] (q-major per partition)."""
        return h.reshape(128, 8 * Lst)

    groups, shapes = plan["groups"], plan["shapes"]
    in_maps = []
    for c in range(NCORES):
        parts = []
        for ch in plan["chunks"]:
            rows = []   # per-partition segments, list of (128, seg) arrays
            for s in ch:
                u = groups[s][c]
                l, b = int(u) // B, int(u) % B
                L1s, L2s = int(shapes[s][0]), int(shapes[s][1])
                b1 = h1t[l, b, :, :L1s]
                b2 = h2t[l, b, :, :L2s]
                if L1s <= L2s:
                    st, mv, Lst = b1, b2, L1s
                else:
                    st, mv, Lst = b2, b1, L2s
                rows.append(swi_pack(st, Lst))
                rows.append(mv.reshape(128, 8 * mv.shape[1]))
            parts.append(np.concatenate(rows, axis=1).ravel())
        in_maps.append({"xin": np.concatenate(parts)})
    return in_maps, len1, len2


def _epilogue(results, len1, len2, w, b, plan, rm_col, cm_col):
    groups, shapes = plan["groups"], plan["shapes"]
    maxv_rows = np.zeros((NL, B, L1), dtype=np.float64)
    maxv_cols = np.zeros((NL, B, L2), dtype=np.float64)
    for c, res in enumerate(results):
        rm = np.asarray(res["rm"], dtype=np.float64)  # (128, nc1)
        cm = np.asarray(res["cm"], dtype=np.float64)
        for s in range(NSLOTS):
            u = groups[s][c]
            l, bb = int(u) // B, int(u) % B
            L1s, L2s = int(shapes[s][0]), int(shapes[s][1])
            for it in range(-(-L1s // 128)):
                ilen = min(128, L1s - it * 128)
                maxv_rows[l, bb, it * 128:it * 128 + ilen] = \
                    rm[:ilen, rm_col[(s, it)]]
            for jt in range(-(-L2s // 128)):
                jlen = min(128, L2s - jt * 128)
                maxv_cols[l, bb, jt * 128:jt * 128 + jlen] = \
                    cm[:jlen, cm_col[(s, jt)]]

    ar1 = np.arange(L1)[None, :]
    ar2 = np.arange(L2)[None, :]
    mask1 = (ar1 < len1[:, None])
    mask2 = (ar2 < len2[:, None])
    n1 = len1.astype(np.float64)
    n2 = len2.astype(np.float64)
    s2 = np.where(mask1[None], maxv_rows, 0.0).sum(axis=2) / n1[None]
    s1 = np.where(mask2[None], maxv_cols, 0.0).sum(axis=2) / n2[None]
    feat = (2.0 * s1 * s2 / (s1 + s2)).T
    mean = feat.mean(axis=0, keepdims=True)
    var = ((feat - mean) ** 2).mean(axis=0, keepdims=True)
    feat = (feat - mean) / np.sqrt(var + BN_EPS)
    w = np.asarray(w, dtype=np.float64)
    bb = np.asarray(b, dtype=np.float64)
    out = LOGIT_SCALE * (feat @ w.T + bb)[:, 0]
    return out.astype(np.float32)


LAST_RUN = {}


def kernel(reps1, reps2, len1, len2, w, b):
    from concourse.bass_utils import run_bass_kernel_spmd

    len1 = np.asarray(len1).astype(np.int64)
    len2 = np.asarray(len2).astype(np.int64)
    plan, nc, rm_col, cm_col = _get_plan_nc(len1, len2)
    in_maps, l1, l2 = _host_prep(reps1, reps2, len1, len2, plan)
    res = run_bass_kernel_spmd(nc, in_maps, list(range(NCORES)))
    LAST_RUN["results"] = res
    LAST_RUN["in_maps"] = in_maps
    LAST_RUN["nc"] = nc
    return _epilogue(res.results, l1, l2, w, b, plan, rm_col, cm_col)


# revision 45
# speedup vs baseline: 1.0789x; 1.0789x over previous
"""BertScore model kernel for Trainium2 (8 NeuronCores, SPMD, length-specialized).

Reference: cosine-normalized per-layer token reps, per-(layer,batch)
similarity matrix dots = h1 @ h2^T (L1 x L2, contraction D=1024), ragged
max over valid rows/cols + means -> s1,s2, F1 harmonic mean -> (B,NL)
features, BatchNorm over batch, linear head -> (B,).

Strategy: the 256 (layer,batch) units are independent until the host-side
BatchNorm. Units are grouped into 32 SPMD "slots" of 8 (one per core) with
similar (len1,len2); the device program is compiled for the actual lengths
(slot shape = max lens over its 8 units, rounded to 16 for the dual-fp8
LDWEIGHTS stride-alignment ISA rule), so only the valid ragged region is
transferred and computed. Host-side replicate-padding (row/col len-1 copied
into the padded tail) keeps every max exact with no masking. Inputs are fp8
e4m3 (end-to-end rel err ~8e-3 vs the 2e-2 gate; DMA is the roofline and
fp8 halves it); matmuls run in DoubleRow perf mode (two 128-deep K-tiles
per instruction, 0.5 cyc/row).

Per slot (stationary = the shorter of h1/h2, halving LDWEIGHTS cost):
4*n_st DoubleRow matmuls -> ACT copies PSUM to SBUF as f16 (both stationary
tiles into one [128,2,L] tile) -> ONE merged DVE free-axis max over the
moving index (both tiles in a single instruction; DVE reduce fixed cost
~165ns dominates the short data phase, so halving the instruction count
matters) -> PE transposes (f16, software-pipelined one slot behind the
matmuls so the in-order PE queue never stalls on the ACT copy) -> one
merged DVE max over the stationary index. Garbage rows in merged-reduce
output columns land only in positions the host epilogue discards. Row/col
max vectors accumulate in SBUF f16 columns; two small DMAs out at the end.
Slots execute largest-first (minimal compute tail) and are DMA'd in
multi-slot chunks (first chunks small for fast pipeline fill) laid out so
each partition reads one contiguous run per chunk (~330 GB/s measured).
Host epilogue: means over valid prefixes, F1, BatchNorm, linear head.

Measured on trn2 (8 cores, NTFF profile of the single NEFF execution):
~44.7-45.2 us (median 45.0) in the final configuration, vs 295 us for the
staged baseline and 131 us for its single-shot profile. Chunk size trades
HWDGE descriptor-generation exposure (~2.6 us fixed per chunk DMA, 128
descriptors) against pipeline granularity; 12KB/partition hides generation
under the ~5 us transfer while keeping the fill acceptable.
"""
import os
import numpy as np

NL, B, L1, L2, D = 4, 64, 256, 256, 1024
NCORES = 8
NUNITS = NL * B           # 256 independent (layer, batch) units
NSLOTS = NUNITS // NCORES  # 32 slots, one unit per core each
KT = D // 128             # contraction tiles
BN_EPS = 1e-8
LOGIT_SCALE = 1.0

CHUNK_BYTES = int(os.environ.get("BSM_CHUNK", str(12 * 1024)))  # per partition

_CACHE = {}


def _plan(len1, len2):
    """Group units into slots; chunk slots for DMA; exec order big-first."""
    l1u = np.repeat(len1[None, :], NL, 0).ravel()   # unit u = l*B + b
    l2u = np.repeat(len2[None, :], NL, 0).ravel()
    nit = -(-l1u // 128)
    key = nit * 10**9 + l2u * 10**3 + l1u
    order_units = np.argsort(-key)                  # big first
    groups = order_units.reshape(NSLOTS, NCORES)
    # dual-fp8 LDWEIGHTS requires 16B-aligned k-tile strides
    shapes = np.stack([
        np.array([-16 * (-l1u[g].max() // 16) for g in groups]),
        np.array([-16 * (-l2u[g].max() // 16) for g in groups])], axis=1)
    order = np.argsort(-(shapes[:, 0] + shapes[:, 1]), kind="stable")
    # chunk consecutive exec-order slots: per-partition run per slot is
    # 8*(L1s+L2s) bytes. First chunks are small so compute starts early
    # (pipeline fill), later ones large (fewer DMA issues).
    ramp = [4 * 1024, 8 * 1024]
    chunks = []
    cur, cur_bytes = [], 0
    for s in order:
        cap = ramp[len(chunks)] if len(chunks) < len(ramp) else CHUNK_BYTES
        run = 8 * int(shapes[s][0] + shapes[s][1])
        if cur and cur_bytes + run > cap:
            chunks.append(cur)
            cur, cur_bytes = [], 0
        cur.append(int(s))
        cur_bytes += run
    if cur:
        chunks.append(cur)
    return {"groups": groups, "shapes": shapes, "order": order,
            "chunks": chunks}


def _build(shapes, chunks, plan):
    import concourse.bacc as bacc
    import concourse.bass as bass
    import concourse.mybir as mybir
    import concourse.tile as tile

    f32 = mybir.dt.float32
    f16 = mybir.dt.float16
    fp8 = mybir.dt.float8e4
    SWI = mybir.MatmulPerfMode.DoubleRowSwInterleave
    DR = mybir.MatmulPerfMode.DoubleRow

    # DRAM layout: chunk-major; within a chunk partition p holds one
    # contiguous run = concat over the chunk's slots of (8 d-rows of h1,
    # 8 d-rows of h2), each slot contributing 8*(L1s+L2s) bytes.
    chunk_off, chunk_run = [], []
    slot_in_chunk = {}
    off = 0
    for ci, ch in enumerate(chunks):
        run = 0
        for s in ch:
            slot_in_chunk[s] = (ci, run)
            run += 8 * int(shapes[s][0] + shapes[s][1])
        chunk_off.append(off)
        chunk_run.append(run)
        off += 128 * run
    TOT = off

    # RM: one f16 column per (slot, it); CM: one per (slot, jt).
    rm_col, cm_col = {}, {}
    nc1 = nc2 = 0
    for ch in chunks:
        for s in ch:
            L1s, L2s = int(shapes[s][0]), int(shapes[s][1])
            for it in range(-(-L1s // 128)):
                rm_col[(s, it)] = nc1
                nc1 += 1
            for jt in range(-(-L2s // 128)):
                cm_col[(s, jt)] = nc2
                nc2 += 1

    nc = bacc.Bacc("TRN2", target_bir_lowering=False, debug=False,
                   num_devices=NCORES)
    xin = nc.dram_tensor("xin", [TOT], fp8, kind="ExternalInput")
    rmd = nc.dram_tensor("rm", [128, nc1], f16, kind="ExternalOutput")
    cmd = nc.dram_tensor("cm", [128, nc2], f16, kind="ExternalOutput")
    xap = xin.ap()

    # quarters: contiguous chunk groups; each gets its own accumulator
    # tiles, DMA'd out as soon as its reduces are done (overlapped drain)
    NQ = 4
    nch = len(chunks)
    chunk_q = [min(NQ - 1, ci * NQ // nch) for ci in range(nch)]
    slot_q = {}
    for ci, ch in enumerate(chunks):
        for s in ch:
            slot_q[s] = chunk_q[ci]
    q_rm, q_cm = {}, {}   # quarter -> (col_lo, col_hi)
    for ci, ch in enumerate(chunks):
        q = chunk_q[ci]
        for s in ch:
            for t in range(-(-int(shapes[s][0]) // 128)):
                c = rm_col[(s, t)]
                lo, hi = q_rm.get(q, (c, c + 1))
                q_rm[q] = (min(lo, c), max(hi, c + 1))
            for t in range(-(-int(shapes[s][1]) // 128)):
                c = cm_col[(s, t)]
                lo, hi = q_cm.get(q, (c, c + 1))
                q_cm[q] = (min(lo, c), max(hi, c + 1))
    last_slot_of_q = {}
    for ci, ch in enumerate(chunks):
        last_slot_of_q[chunk_q[ci]] = ch[-1]
    last_slot_q = {s: q for q, s in last_slot_of_q.items()}

    with tile.TileContext(nc) as tc:
        from concourse.masks import make_identity
        with tc.tile_pool(name="consts", bufs=1) as consts, \
             tc.tile_pool(name="io", bufs=6) as io, \
             tc.tile_pool(name="dsbp", bufs=6) as dsbp, \
             tc.tile_pool(name="accp", bufs=1) as accp, \
             tc.tile_pool(name="ps", bufs=4, space="PSUM") as ps, \
             tc.tile_pool(name="psT", bufs=4, space="PSUM") as psT:

            ident = consts.tile([128, 128], f16)
            make_identity(nc, ident)
            RM = accp.tile([128, nc1], f16)
            CM = accp.tile([128, nc2], f16)

            vmax = mybir.AluOpType.max
            X = mybir.AxisListType.X
            IDENT = mybir.ActivationFunctionType.Identity

            MAXRUN = max(chunk_run)

            def emit_transposes(st):
                """Transpose phase of a slot: dT blocks + moving-side max."""
                s, Lst, Lmv, dsb, ilens, tr_acc, lc = st
                n_mv = -(-Lmv // 128)
                dT = psT.tile([128, 2, L1], f16, tag="dT")
                for tt in range(n_mv):
                    j0 = tt * 128
                    jlen = min(128, Lmv - j0)
                    for t, ilen in enumerate(ilens):
                        nc.tensor.transpose(
                            out=dT[:jlen, tt, t * 128:t * 128 + ilen],
                            in_=dsb[:ilen, t, j0:j0 + jlen],
                            identity=ident[:ilen, :ilen])
                if n_mv == 2:
                    # one merged reduce; tt=1 rows past jlen are garbage and
                    # discarded host-side (reduction is per-partition)
                    nc.vector.tensor_reduce(
                        out=tr_acc[:, lc:lc + 2], in_=dT[:, :, :Lst],
                        axis=X, op=vmax)
                else:
                    jlen = min(128, Lmv)
                    nc.vector.tensor_reduce(
                        out=tr_acc[:jlen, lc:lc + 1], in_=dT[:jlen, 0, :Lst],
                        axis=X, op=vmax)

            pending = None   # software pipeline: transposes lag one slot
            for ci, ch in enumerate(chunks):
                blk = io.tile([128, MAXRUN], fp8, tag="io")
                run = chunk_run[ci]
                # overlap the first two descriptor generations: ACT is idle
                # during pipeline fill and has its own HWDGE queue
                deng = nc.scalar if ci == 1 else nc.sync
                deng.dma_start(
                    out=blk[:, :run],
                    in_=bass.AP(tensor=xap.tensor, offset=chunk_off[ci],
                                ap=[[run, 128], [1, run]]))
                for s in ch:
                    L1s, L2s = int(shapes[s][0]), int(shapes[s][1])
                    soff = slot_in_chunk[s][1]
                    # stationary = shorter side (LDWEIGHTS cost ~ 8*Lst);
                    # free-axis reduce covers the stationary index,
                    # transposes cover the moving index.
                    if L1s <= L2s:
                        Lst, Lmv = L1s, L2s
                        fr_acc, frc = RM, rm_col[(s, 0)]
                        tr_acc, trc = CM, cm_col[(s, 0)]
                    else:
                        Lst, Lmv = L2s, L1s
                        fr_acc, frc = CM, cm_col[(s, 0)]
                        tr_acc, trc = RM, rm_col[(s, 0)]
                    stv = blk[:, soff:soff + 8 * Lst].rearrange(
                        "p (q i) -> p q i", q=8)
                    mvv = blk[:, soff + 8 * Lst:soff + 8 * (Lst + Lmv)
                              ].rearrange("p (q j) -> p q j", q=8)
                    n_st = -(-Lst // 128)

                    dsb = dsbp.tile([128, 2, L2], f16, tag="dsb")
                    ilens = []
                    for t in range(n_st):
                        i0 = t * 128
                        ilen = min(128, Lst - i0)
                        dps = ps.tile([128, L2], f32, tag="dots")
                        for k in range(0, KT, 2):
                            nc.tensor.matmul(
                                out=dps[:ilen, :Lmv],
                                lhsT=stv[:, k:k + 2, i0:i0 + ilen],
                                rhs=mvv[:, k:k + 2, :],
                                start=(k == 0), stop=(k == KT - 2),
                                perf_mode=DR)
                        nc.scalar.activation(
                            out=dsb[:ilen, t, :Lmv], in_=dps[:ilen, :Lmv],
                            func=IDENT)
                        ilens.append(ilen)
                    if n_st == 2:
                        # merged reduce; tile-1 rows past its ilen are
                        # garbage and discarded host-side
                        nc.vector.tensor_reduce(
                            out=fr_acc[:, frc:frc + 2],
                            in_=dsb[:, :, :Lmv], axis=X, op=vmax)
                    else:
                        nc.vector.tensor_reduce(
                            out=fr_acc[:ilens[0], frc:frc + 1],
                            in_=dsb[:ilens[0], 0, :Lmv], axis=X, op=vmax)

                    if pending is not None:
                        emit_transposes(pending)
                    pending = (s, Lst, Lmv, dsb, ilens, tr_acc, trc)
            if pending is not None:
                emit_transposes(pending)
            nc.sync.dma_start(out=rmd.ap(), in_=RM)
            nc.scalar.dma_start(out=cmd.ap(), in_=CM)

    nc.finalize()
    return nc, rm_col, cm_col


def _get_plan_nc(len1, len2):
    key = (tuple(len1.tolist()), tuple(len2.tolist()), CHUNK_BYTES)
    if key not in _CACHE:
        plan = _plan(len1, len2)
        nc, rm_col, cm_col = _build(plan["shapes"], plan["chunks"], plan)
        _CACHE[key] = (plan, nc, rm_col, cm_col)
    return _CACHE[key]


def _host_prep(reps1, reps2, len1, len2, plan):
    """Normalize, fp8-cast, replicate-pad, pack per-core chunk buffers."""
    import ml_dtypes
    np_in = ml_dtypes.float8_e4m3

    def prep(r, lens, L):
        r = np.asarray(r, dtype=np.float32)
        n = np.sqrt(np.einsum('lbid,lbid->lbi', r, r))
        h = r / n[..., None]
        idx = np.minimum(np.arange(L)[None, :], (lens - 1)[:, None])  # (B, L)
        h = np.take_along_axis(h, idx[None, :, :, None], axis=2)
        return np.ascontiguousarray(h.transpose(0, 1, 3, 2)).astype(np_in)

    h1t = prep(reps1, len1, L1)   # (NL, B, D, L)
    h2t = prep(reps2, len2, L2)

    def swi_pack(h, Lst):
        """[1024, Lst] d-major -> [128, 8*Lst] (q-major per partition)."""
        return h.reshape(128, 8 * Lst)

    groups, shapes = plan["groups"], plan["shapes"]
    in_maps = []
    for c in range(NCORES):
        parts = []
        for ch in plan["chunks"]:
            rows = []   # per-partition segments, list of (128, seg) arrays
            for s in ch:
                u = groups[s][c]
                l, b = int(u) // B, int(u) % B
                L1s, L2s = int(shapes[s][0]), int(shapes[s][1])
                b1 = h1t[l, b, :, :L1s]
                b2 = h2t[l, b, :, :L2s]
                if L1s <= L2s:
                    st, mv, Lst = b1, b2, L1s
                else:
                    st, mv, Lst = b2, b1, L2s
                rows.append(swi_pack(st, Lst))
                rows.append(mv.reshape(128, 8 * mv.shape[1]))
            parts.append(np.concatenate(rows, axis=1).ravel())
        in_maps.append({"xin": np.concatenate(parts)})
    return in_maps, len1, len2


def _epilogue(results, len1, len2, w, b, plan, rm_col, cm_col):
    groups, shapes = plan["groups"], plan["shapes"]
    maxv_rows = np.zeros((NL, B, L1), dtype=np.float64)
    maxv_cols = np.zeros((NL, B, L2), dtype=np.float64)
    for c, res in enumerate(results):
        rm = np.asarray(res["rm"], dtype=np.float64)  # (128, nc1)
        cm = np.asarray(res["cm"], dtype=np.float64)
        for s in range(NSLOTS):
            u = groups[s][c]
            l, bb = int(u) // B, int(u) % B
            L1s, L2s = int(shapes[s][0]), int(shapes[s][1])
            for it in range(-(-L1s // 128)):
                ilen = min(128, L1s - it * 128)
                maxv_rows[l, bb, it * 128:it * 128 + ilen] = \
                    rm[:ilen, rm_col[(s, it)]]
            for jt in range(-(-L2s // 128)):
                jlen = min(128, L2s - jt * 128)
                maxv_cols[l, bb, jt * 128:jt * 128 + jlen] = \
                    cm[:jlen, cm_col[(s, jt)]]

    ar1 = np.arange(L1)[None, :]
    ar2 = np.arange(L2)[None, :]
    mask1 = (ar1 < len1[:, None])
    mask2 = (ar2 < len2[:, None])
    n1 = len1.astype(np.float64)
    n2 = len2.astype(np.float64)
    s2 = np.where(mask1[None], maxv_rows, 0.0).sum(axis=2) / n1[None]
    s1 = np.where(mask2[None], maxv_cols, 0.0).sum(axis=2) / n2[None]
    feat = (2.0 * s1 * s2 / (s1 + s2)).T
    mean = feat.mean(axis=0, keepdims=True)
    var = ((feat - mean) ** 2).mean(axis=0, keepdims=True)
    feat = (feat - mean) / np.sqrt(var + BN_EPS)
    w = np.asarray(w, dtype=np.float64)
    bb = np.asarray(b, dtype=np.float64)
    out = LOGIT_SCALE * (feat @ w.T + bb)[:, 0]
    return out.astype(np.float32)


LAST_RUN = {}


def kernel(reps1, reps2, len1, len2, w, b):
    from concourse.bass_utils import run_bass_kernel_spmd

    len1 = np.asarray(len1).astype(np.int64)
    len2 = np.asarray(len2).astype(np.int64)
    plan, nc, rm_col, cm_col = _get_plan_nc(len1, len2)
    in_maps, l1, l2 = _host_prep(reps1, reps2, len1, len2, plan)
    res = run_bass_kernel_spmd(nc, in_maps, list(range(NCORES)))
    LAST_RUN["results"] = res
    LAST_RUN["in_maps"] = in_maps
    LAST_RUN["nc"] = nc
    return _epilogue(res.results, l1, l2, w, b, plan, rm_col, cm_col)
